# revision 4
# baseline (speedup 1.0000x reference)
"""DiffAttn Trainium2 Bass kernel, v2: replicate-K / gather-V design.

Each core handles (batch b, q-block qb) = divmod(core, 4): 1024 q rows.
It streams the FULL batch xT (16MB) and projects kT for ALL 4096 keys
locally (scores never wait on a collective), projects V for its OWN
1024-row block, and AllGathers the V parts (1MB out, ~41us modeled,
hidden under scores/exp).

Layout: scores TRANSPOSED ([sk, q], keys on partitions) so exp(scores)
feeds PV directly with contraction over sk. PV uses e-subtiles as the
STATIONARY operand (out [q,128pt x h] natural orientation, cost = moving
h-cols = same as before, but U needs no post transpose) and row-sums
ride the same stationaries with a ones moving vector (out free-size 1 =
~free in the cost model, killing the old ones-matmul pass entirely).

PV/sums for ALL blocks are deferred until the V gather lands (~49us);
e tiles are buffered in SBUF meanwhile (group 0 worst case ~35 tiles).
"""

import math
import os
import sys
from contextlib import ExitStack

import numpy as np

for _p in ("/root/.axon_site/_ro/trn_rl_repo", "/opt/trn_rl_repo"):
    if os.path.isdir(_p) and _p not in sys.path:
        sys.path.append(_p)

import ml_dtypes  # noqa: E402

import concourse.bass as bass  # noqa: E402
import concourse.mybir as mybir  # noqa: E402
import concourse.tile as tile  # noqa: E402
from concourse import bacc, bass_utils  # noqa: E402

B, S, D, H = 2, 4096, 2048, 128
H2 = H // 2  # 64
P = 128
NCORES = 8
QSHARD = 1024  # q rows per core
DCH = D // P  # 16 d-chunks
NKCH = S // P  # 32 key chunks of 128
NBLK, BLKW = 4, 1024  # key blocks
NGROUPS, GW = 2, 512  # q groups per core
NJ = GW // P  # 4 q sub-blocks of 128 per group

LAMBDA_INIT = 0.8 - 0.6 * math.exp(-0.3 * 12)
RMS_EPS = float(np.finfo(np.float32).eps)
SCALE = 1.0 / math.sqrt(H2)

F32 = mybir.dt.float32
BF16 = mybir.dt.bfloat16

AF = mybir.ActivationFunctionType
OP = mybir.AluOpType


def _emit(ctx: ExitStack, tc: "tile.TileContext", lam: float):
    nc = tc.nc

    # xT: 5 blocks of [D, 1024] bf16. Position 0 = OWN block (feeds V and q
    # projections); positions 1..4 = the batch's blocks 0..3 in batch order
    # (feed kT, whose chunk index must match the V AllGather rank order).
    # The own block is duplicated (one extra hidden 4MB DMA) so the SPMD
    # program needs no per-core knowledge of its block index.
    xT = nc.dram_tensor("xT", (NBLK + 1, D, BLKW), BF16, kind="ExternalInput").ap()
    # weights pre-arranged on host as [p, c, h] so the DMA rows are 4KB
    # contiguous (sub-512B elements pay a 2x DMA latency penalty)
    wqT = nc.dram_tensor("wqT", (P, DCH, H), BF16, kind="ExternalInput").ap()
    wkT = nc.dram_tensor("wkT", (P, DCH, H), BF16, kind="ExternalInput").ap()
    wvT = nc.dram_tensor("wvT", (P, DCH, H), BF16, kind="ExternalInput").ap()
    rmsw = nc.dram_tensor("rmsw", (H,), F32, kind="ExternalInput").ap()
    part_d = nc.dram_tensor("part_d", (P, BLKW), BF16).ap()
    full_d = nc.dram_tensor("full_d", (NBLK, P, BLKW), BF16).ap()
    out_d = nc.dram_tensor("out", (QSHARD, H), F32, kind="ExternalOutput").ap()

    consts = ctx.enter_context(tc.tile_pool(name="consts", bufs=1))
    persist = ctx.enter_context(tc.tile_pool(name="persist", bufs=1))

    ones_bf = consts.tile([P, 1], BF16)
    nc.vector.memset(ones_bf, 1.0)
    rmsw_bc = consts.tile([P, H], F32)
    nc.sync.dma_start(
        out=rmsw_bc,
        in_=bass.AP(tensor=rmsw.tensor, offset=0, ap=[[0, P], [1, H]]),
    )
    wq_sb = consts.tile([P, DCH, H], BF16)
    wk_sb = consts.tile([P, DCH, H], BF16)
    wv_sb = consts.tile([P, DCH, H], BF16)

    kT_sb = persist.tile([P, S], BF16)  # [h, sk] all 4 blocks, batch order
    qT_sb = persist.tile([P, QSHARD], BF16)  # [h, sq]
    vpart_sb = persist.tile([P, 8, P], BF16)  # own block V [sk%128, ch, h]
    v_sb = persist.tile([P, NKCH, P], BF16)  # gathered V, batch order

    xr = xT.rearrange("b (c p) w -> b p c w", p=P)
    epool = ctx.enter_context(tc.tile_pool(name="epool", bufs=52))
    small = ctx.enter_context(tc.tile_pool(name="small", bufs=8))
    outp = ctx.enter_context(tc.tile_pool(name="outp", bufs=4))
    usb_pool = ctx.enter_context(tc.tile_pool(name="usb", bufs=1))

    # scores PSUM pool first so it does NOT share banks with (and thus wait
    # on) the projection pool: 4 + 3 banks coexist, u/sum pools reuse the
    # proj banks after release (they wait for the V gather anyway).
    pp_s = ctx.enter_context(tc.tile_pool(name="pp_s", space="PSUM", bufs=2))

    # interleaved (g0, ch)(g1, ch) score+exp pair emission, fed just-in-time
    # as kT blocks complete so PE FIFO order never head-of-line blocks ACT
    es = {g: [] for g in range(NGROUPS)}
    # group 0 leads group 1 by LEAD chunks so the very first exps never
    # head-of-line block on q1's projection (which follows the first kacc)
    LEAD = 1
    SCHED = [(0, c) for c in range(LEAD)]
    for c in range(NKCH):
        if c + LEAD < NKCH:
            SCHED.append((0, c + LEAD))
        SCHED.append((1, c))
    _cursor = [0]  # position in SCHED (2 entries per chunk overall)

    def emit_one(g, ch):
        q0 = g * GW
        s_ps = pp_s.tile([P, 2 * GW], F32, tag="s", name="s_ps")
        nc.tensor.matmul(
            s_ps[:, 0:GW],
            kT_sb[0:H2, ch * P : (ch + 1) * P],
            qT_sb[0:H2, q0 : q0 + GW],
        )
        nc.tensor.matmul(
            s_ps[:, GW : 2 * GW],
            kT_sb[H2:H, ch * P : (ch + 1) * P],
            qT_sb[H2:H, q0 : q0 + GW],
        )
        e_sb = epool.tile([P, 2 * GW], BF16, tag="e", name="e_sb")
        nc.scalar.activation(e_sb, s_ps, AF.Exp, scale=SCALE)
        es[g].append(e_sb)

    def emit_pairs(limit_ch, maxn):
        # maxn counts chunk-equivalents (2 SCHED entries)
        n = 2 * maxn
        while _cursor[0] < len(SCHED) and n > 0:
            g, ch = SCHED[_cursor[0]]
            if ch >= limit_ch:
                break
            emit_one(g, ch)
            _cursor[0] += 1
            n -= 1

    # ---- projections (x streamed as half-block tiles for tight deps) ----
    # DMA order: x0h0, wv, x0h1, wk, wq, x1h0, x1h1, x2... so the V->collective
    # gate (x0+wv) and the first-kT gate (wk+x1h0) resolve as early as possible.
    with tc.tile_pool(name="xstream", bufs=3) as xpool, \
         tc.tile_pool(name="pp_proj", space="PSUM", bufs=1) as pp_proj:

        def load_xh(pos, sl, quarters=False):
            xh = xpool.tile([P, DCH, 512], BF16, tag="x", name=f"x{pos}_{sl}")
            if quarters:
                for qq in range(4):
                    nc.sync.dma_start(
                        out=xh[:, qq * 4 : (qq + 1) * 4, :],
                        in_=xr[pos, :, qq * 4 : (qq + 1) * 4,
                               sl * 512 : (sl + 1) * 512],
                    )
            else:
                nc.sync.dma_start(
                    out=xh, in_=xr[pos, :, :, sl * 512 : (sl + 1) * 512]
                )
            return xh

        nc.sync.dma_start(out=wv_sb, in_=wvT)
        x0h = [load_xh(0, 0, quarters=True)]
        nc.sync.dma_start(out=wq_sb, in_=wqT)
        x0h.append(load_xh(0, 1, quarters=True))
        nc.sync.dma_start(out=wk_sb, in_=wkT)

        # PE p-state warm-up: the tensor engine only reaches 2.4GHz after
        # 3us of CONTINUOUS execution, and any idle gap resets the ramp.
        # Run discarded matmuls from t~0 so the V projection (which gates
        # the collective) starts at full clock with the ramp already paid.
        dummy_sb = consts.tile([P, 512], BF16)
        nc.vector.memset(dummy_sb, 0.0)

        def warm(n):
            s_dummy = pp_s.tile([P, 2 * GW], F32, tag="s", name="warm")
            for _ in range(n):
                nc.tensor.matmul(s_dummy[0:1, 0:512], ones_bf, dummy_sb)


        # V (own block): stationary x chunk [d,128s], moving wv [d,128h].
        # Both V halves back-to-back: the AllGather dispatch is the kernel's
        # longest gate (collective ~41us + B-phase ~28us behind it).
        def proj_q(hf2):
            qacc = pp_proj.tile([P, 512], F32, tag="kacc", bufs=2)
            for c in range(DCH):
                nc.tensor.matmul(
                    qacc, wq_sb[:, c, :], x0h[hf2][:, c, :],
                    start=(c == 0), stop=(c == DCH - 1),
                )
            nc.vector.tensor_copy(qT_sb[:, hf2 * 512 : (hf2 + 1) * 512], qacc)

        for hf in range(2):
            vacc = pp_proj.tile([P, 4, P], F32, tag="vacc", bufs=1)
            for j4 in range(4):
                for c in range(DCH):
                    nc.tensor.matmul(
                        vacc[:, j4, :],
                        x0h[hf][:, c, j4 * P : (j4 + 1) * P],
                        wv_sb[:, c, :],
                        start=(c == 0),
                        stop=(c == DCH - 1),
                    )
            nc.vector.tensor_copy(vpart_sb[:, hf * 4 : (hf + 1) * 4, :], vacc)
            if hf == 0:
                proj_q(0)  # fills the PE gap while x0h1 streams in
        nc.sync.dma_start(out=part_d, in_=vpart_sb.rearrange("p j h -> p (j h)"))
        nc.gpsimd.collective_compute(
            "AllGather",
            OP.bypass,
            replica_groups=[[0, 1, 2, 3], [4, 5, 6, 7]],
            ins=[part_d.opt()],
            outs=[full_d.opt()],
        )


        # kT for batch blocks 0..3 (stream positions 1..4) with score/exp
        # pairs interleaved as coverage grows
        for blk in range(NBLK):
            for sl in range(2):
                xh = load_xh(blk + 1, sl, quarters=(blk == 0))
                kacc = pp_proj.tile([P, 512], F32, tag="kacc", bufs=2)
                for c in range(DCH):
                    nc.tensor.matmul(
                        kacc, wk_sb[:, c, :], xh[:, c, :],
                        start=(c == 0), stop=(c == DCH - 1),
                    )
                nc.vector.tensor_copy(
                    kT_sb[:, blk * BLKW + sl * 512 : blk * BLKW + (sl + 1) * 512],
                    kacc,
                )
                if blk == 0 and sl == 0:
                    proj_q(1)
                emit_pairs(blk * 8 + (sl + 1) * 4, 5)

    # gathered V -> SBUF (batch order: host maps rank r -> block order)
    for r in range(NBLK):
        nc.sync.dma_start(
            out=v_sb[:, r * 8 : (r + 1) * 8, :],
            in_=full_d[r].rearrange("p (j h) -> p j h", j=8),
        )

    pp_u = ctx.enter_context(tc.tile_pool(name="pp_u", space="PSUM", bufs=1))
    pp_sum = ctx.enter_context(tc.tile_pool(name="pp_sum", space="PSUM", bufs=2))

    # ---- attention ----
    # Phase A (both groups back-to-back so ACT never starves): scores + exp
    # for all 64 chunk-visits; e is buffered in SBUF.
    # Phase B (per group, after the V gather lands): PV + sums bursts via
    # e-stationary matmuls (sums outputs have free-size 1 => ~free).
    c_ = 1.0 - LAMBDA_INIT
    a_ = 1.0 / (H * c_ * c_)
    b_ = RMS_EPS / (c_ * c_)
    bias_b = consts.tile([P, 1], F32)
    nc.vector.memset(bias_b, b_)

    def bcast_h(t, j0):
        """[P,1] columns j0..j0+NJ of t broadcast along h: [P, NJ, P]"""
        base = t.opt()
        return bass.AP(
            tensor=t.tensor,
            offset=base.offset + j0,
            ap=[list(base.ap[0]), [1, NJ], [0, P]],
        )

    def post_group(g, u_ap, sums_ps):
        """normalize + combine + RMS + store for one group (u_ap may be
        PSUM or SBUF). Batched across the NJ q-subtiles."""
        r_sb = small.tile([P, 2 * NJ], F32, tag="r", bufs=2)
        nc.vector.reciprocal(r_sb, sums_ps)
        nc.vector.tensor_scalar_mul(r_sb[:, NJ:], r_sb[:, NJ:], lam)
        # attn = u1 * r1 - u2 * r2  (r broadcast along h via stride-0 AP)
        t2 = small.tile([P, NJ, P], F32, tag="t2", bufs=2)
        nc.vector.tensor_tensor(
            t2, u_ap[:, NJ : 2 * NJ, :], bcast_h(r_sb, NJ), op=OP.mult
        )
        t1 = small.tile([P, NJ, P], F32, tag="t1", bufs=2)
        nc.vector.tensor_tensor(
            t1, u_ap[:, 0:NJ, :], bcast_h(r_sb, 0), op=OP.mult
        )
        attn_sb = outp.tile([P, NJ, P], F32, tag="attn", bufs=2)
        nc.vector.tensor_tensor(attn_sb, t1, t2, op=OP.subtract)
        # RMS stats: per-j accumulated square sums into one [P, NJ] tile
        ssq = small.tile([P, NJ], F32, tag="ssq", bufs=2)
        sq_scr = small.tile([P, NJ, P], F32, tag="sqscr", bufs=2)
        for j in range(NJ):
            nc.scalar.activation(
                sq_scr[:, j, :], attn_sb[:, j, :], AF.Square,
                accum_out=ssq[:, j : j + 1],
            )
        root = small.tile([P, NJ], F32, tag="root", bufs=2)
        nc.scalar.activation(root, ssq, AF.Sqrt, scale=a_, bias=bias_b)
        rrms = small.tile([P, NJ], F32, tag="rrms", bufs=2)
        nc.vector.reciprocal(rrms, root)
        # batched store: o[p, j, h] = attn * rrms (bcast h) * rmsw (bcast j),
        # one wide DVE chain and ONE output DMA per group
        o_sb = outp.tile([P, NJ, H], F32, tag="o", bufs=1)
        nc.vector.tensor_tensor(o_sb, attn_sb, bcast_h(rrms, 0), op=OP.mult)
        rmsw4 = bass.AP(
            tensor=rmsw_bc.tensor,
            offset=rmsw_bc.opt().offset,
            ap=[list(rmsw_bc.opt().ap[0]), [0, NJ], [1, H]],
        )
        nc.vector.tensor_tensor(o_sb, o_sb, rmsw4, op=OP.mult)
        nc.sync.dma_start(
            out=bass.AP(
                tensor=out_d.tensor,
                offset=g * GW * H,
                ap=[[H, P], [P * H, NJ], [1, H]],
            ),
            in_=o_sb,
        )

    # Phase B. ch-outer / hj-inner with ONE accumulation group per 2KB psum
    # zero region: u bank0 = cells hj 0..3, u bank1 = hj 4..7, sums = all 8
    # cells in one bank. start fires on the region's first write, stop on
    # its last. B(g0) chunks are interleaved into the late score pairs (see
    # emit loop below) so they don't serialize behind the score tail.
    b_tiles = {}

    def get_b_tiles(g):
        if g not in b_tiles:
            u_t = pp_u.tile([P, 2 * NJ, P], F32, tag="u", name=f"u{g}")
            sums_t = pp_sum.tile([P, 2 * NJ], F32, tag="sum", name=f"sums{g}")
            b_tiles[g] = (u_t, sums_t)
        return b_tiles[g]

    def emit_B(g, ch):
        u_ps, sums_ps = get_b_tiles(g)
        for hj in range(2 * NJ):
            e_sub = es[g][ch][:, hj * P : (hj + 1) * P]
            nc.tensor.matmul(
                u_ps[:, hj, :], e_sub, v_sb[:, ch, :],
                start=(ch == 0 and hj % NJ == 0),
                stop=(ch == NKCH - 1 and hj % NJ == NJ - 1),
                skip_group_check=True,
            )
            nc.tensor.matmul(
                sums_ps[:, hj : hj + 1], e_sub, ones_bf,
                start=(ch == 0 and hj == 0),
                stop=(ch == NKCH - 1 and hj == 2 * NJ - 1),
                skip_group_check=True,
            )

    bcur = [0, 0]
    g0_closed = [False]

    def close_g0():
        # copy u0 out (frees the psum banks for group 1) and emit g0's post
        u0_ps, sums0_ps = get_b_tiles(0)
        u_sb = usb_pool.tile([P, 2 * NJ, P], F32, tag="usb")
        nc.vector.tensor_copy(u_sb, u0_ps)
        post_group(0, u_sb, sums0_ps)
        g0_closed[0] = True

    def maybe_B(n):
        # inline PV/sums: group 0 first, then hand the psum banks to group 1
        while n > 0 and _cursor[0] >= 38:
            if bcur[0] < NKCH:
                if bcur[0] >= len(es[0]):
                    break
                emit_B(0, bcur[0])
                bcur[0] += 1
            else:
                if not g0_closed[0]:
                    close_g0()
                if bcur[1] >= len(es[1]):
                    break
                emit_B(1, bcur[1])
                bcur[1] += 1
            n -= 1

    while _cursor[0] < len(SCHED):
        emit_pairs(NKCH, 1)
        maybe_B(4)
    while bcur[0] < NKCH:
        emit_B(0, bcur[0])
        bcur[0] += 1
    if not g0_closed[0]:
        close_g0()
    while bcur[1] < NKCH:
        emit_B(1, bcur[1])
        bcur[1] += 1
    u1_ps, sums1_ps = get_b_tiles(1)
    post_group(1, u1_ps, sums1_ps)


def build(lam: float):
    from concourse._compat import axon_active

    nc = bacc.Bacc(
        "TRN2",
        target_bir_lowering=False,
        debug=not axon_active(),
        num_devices=NCORES,
    )
    with tile.TileContext(nc) as tc:
        with ExitStack() as ctx:
            _emit(ctx, tc, lam)
    nc.compile()
    return nc


def make_in_maps(x, Wq, Wk, Wv, rms_weight):
    bf = ml_dtypes.bfloat16
    x = np.asarray(x, dtype=np.float32)
    xT = np.ascontiguousarray(x.transpose(0, 2, 1)).astype(bf)  # [B, D, S]
    def warr(W):
        # [P, DCH, H]: warr[p, c, h] = W.T[c*128 + p, h]
        wT = np.asarray(W, np.float32).T.reshape(DCH, P, H)
        return np.ascontiguousarray(wT.transpose(1, 0, 2)).astype(bf)

    wqT, wkT, wvT = warr(Wq), warr(Wk), warr(Wv)
    rw = np.ascontiguousarray(np.asarray(rms_weight, np.float32))
    in_maps = []
    for core in range(NCORES):
        b, qb = divmod(core, NCORES // B)
        # 5 xT blocks: position 0 = own block (feeds V/q projections),
        # positions 1..4 = batch blocks 0..3 (feed kT in batch order, which
        # matches the V AllGather rank order). The duplicate own block keeps
        # the SPMD program free of per-core block-index knowledge.
        blocks = [xT[b][:, qb * BLKW : (qb + 1) * BLKW]]
        for r in range(NBLK):
            blocks.append(xT[b][:, r * BLKW : (r + 1) * BLKW])
        in_maps.append(
            {
                "xT": np.ascontiguousarray(np.stack(blocks)),
                "wqT": wqT,
                "wkT": wkT,
                "wvT": wvT,
                "rmsw": rw,
            }
        )
    return in_maps


def kernel(x, Wq, Wk, Wv, lambda_q1, lambda_q2, lambda_k1, lambda_k2, rms_weight):
    lq1 = np.asarray(lambda_q1, np.float32)
    lq2 = np.asarray(lambda_q2, np.float32)
    lk1 = np.asarray(lambda_k1, np.float32)
    lk2 = np.asarray(lambda_k2, np.float32)
    lam = float(np.exp(np.dot(lq1, lk1)) - np.exp(np.dot(lq2, lk2)) + LAMBDA_INIT)
    nc = build(lam)
    in_maps = make_in_maps(x, Wq, Wk, Wv, rms_weight)
    res = bass_utils.run_bass_kernel_spmd(nc, in_maps, core_ids=list(range(NCORES)))
    out = np.empty((B, S, H), np.float32)
    for core in range(NCORES):
        b, qb = divmod(core, NCORES // B)
        out[b, qb * QSHARD : (qb + 1) * QSHARD] = res.results[core]["out"]
    return out


# revision 5
# speedup vs baseline: 1.0022x; 1.0022x over previous
"""DiffAttn Trainium2 Bass kernel, v2: replicate-K / gather-V design.

Each core handles (batch b, q-block qb) = divmod(core, 4): 1024 q rows.
It streams the FULL batch xT (16MB) and projects kT for ALL 4096 keys
locally (scores never wait on a collective), projects V for its OWN
1024-row block, and AllGathers the V parts (1MB out, ~41us modeled,
hidden under scores/exp).

Layout: scores TRANSPOSED ([sk, q], keys on partitions) so exp(scores)
feeds PV directly with contraction over sk. PV uses e-subtiles as the
STATIONARY operand (out [q,128pt x h] natural orientation, cost = moving
h-cols = same as before, but U needs no post transpose) and row-sums
ride the same stationaries with a ones moving vector (out free-size 1 =
~free in the cost model, killing the old ones-matmul pass entirely).

PV/sums for ALL blocks are deferred until the V gather lands (~49us);
e tiles are buffered in SBUF meanwhile (group 0 worst case ~35 tiles).
"""

import math
import os
import sys
from contextlib import ExitStack

import numpy as np

for _p in ("/root/.axon_site/_ro/trn_rl_repo", "/opt/trn_rl_repo"):
    if os.path.isdir(_p) and _p not in sys.path:
        sys.path.append(_p)

import ml_dtypes  # noqa: E402

import concourse.bass as bass  # noqa: E402
import concourse.mybir as mybir  # noqa: E402
import concourse.tile as tile  # noqa: E402
from concourse import bacc, bass_utils  # noqa: E402

B, S, D, H = 2, 4096, 2048, 128
H2 = H // 2  # 64
P = 128
NCORES = 8
QSHARD = 1024  # q rows per core
DCH = D // P  # 16 d-chunks
NKCH = S // P  # 32 key chunks of 128
NBLK, BLKW = 4, 1024  # key blocks
NGROUPS, GW = 2, 512  # q groups per core
NJ = GW // P  # 4 q sub-blocks of 128 per group

LAMBDA_INIT = 0.8 - 0.6 * math.exp(-0.3 * 12)
RMS_EPS = float(np.finfo(np.float32).eps)
SCALE = 1.0 / math.sqrt(H2)

F32 = mybir.dt.float32
BF16 = mybir.dt.bfloat16

AF = mybir.ActivationFunctionType
OP = mybir.AluOpType


def _emit(ctx: ExitStack, tc: "tile.TileContext", lam: float):
    nc = tc.nc

    # xT: 5 blocks of [D, 1024] bf16. Position 0 = OWN block (feeds V and q
    # projections); positions 1..4 = the batch's blocks 0..3 in batch order
    # (feed kT, whose chunk index must match the V AllGather rank order).
    # The own block is duplicated (one extra hidden 4MB DMA) so the SPMD
    # program needs no per-core knowledge of its block index.
    xT = nc.dram_tensor("xT", (NBLK + 1, D, BLKW), BF16, kind="ExternalInput").ap()
    # weights pre-arranged on host as [p, c, h] so the DMA rows are 4KB
    # contiguous (sub-512B elements pay a 2x DMA latency penalty)
    wqT = nc.dram_tensor("wqT", (P, DCH, H), BF16, kind="ExternalInput").ap()
    wkT = nc.dram_tensor("wkT", (P, DCH, H), BF16, kind="ExternalInput").ap()
    wvT = nc.dram_tensor("wvT", (P, DCH, H), BF16, kind="ExternalInput").ap()
    rmsw = nc.dram_tensor("rmsw", (H,), F32, kind="ExternalInput").ap()
    part_d = nc.dram_tensor("part_d", (P, BLKW), BF16).ap()
    full_d = nc.dram_tensor("full_d", (NBLK, P, BLKW), BF16).ap()
    out_d = nc.dram_tensor("out", (QSHARD, H), F32, kind="ExternalOutput").ap()

    consts = ctx.enter_context(tc.tile_pool(name="consts", bufs=1))
    persist = ctx.enter_context(tc.tile_pool(name="persist", bufs=1))

    ones_bf = consts.tile([P, 1], BF16)
    nc.vector.memset(ones_bf, 1.0)
    rmsw_bc = consts.tile([P, H], F32)
    nc.sync.dma_start(
        out=rmsw_bc,
        in_=bass.AP(tensor=rmsw.tensor, offset=0, ap=[[0, P], [1, H]]),
    )
    wq_sb = consts.tile([P, DCH, H], BF16)
    wk_sb = consts.tile([P, DCH, H], BF16)
    wv_sb = consts.tile([P, DCH, H], BF16)

    kT_sb = persist.tile([P, S], BF16)  # [h, sk] all 4 blocks, batch order
    qT_sb = persist.tile([P, QSHARD], BF16)  # [h, sq]
    vpart_sb = persist.tile([P, 8, P], BF16)  # own block V [sk%128, ch, h]
    v_sb = persist.tile([P, NKCH, P], BF16)  # gathered V, batch order

    xr = xT.rearrange("b (c p) w -> b p c w", p=P)
    epool = ctx.enter_context(tc.tile_pool(name="epool", bufs=52))
    small = ctx.enter_context(tc.tile_pool(name="small", bufs=8))
    outp = ctx.enter_context(tc.tile_pool(name="outp", bufs=4))
    usb_pool = ctx.enter_context(tc.tile_pool(name="usb", bufs=1))

    # scores PSUM pool first so it does NOT share banks with (and thus wait
    # on) the projection pool: 4 + 3 banks coexist, u/sum pools reuse the
    # proj banks after release (they wait for the V gather anyway).
    pp_s = ctx.enter_context(tc.tile_pool(name="pp_s", space="PSUM", bufs=2))

    # interleaved (g0, ch)(g1, ch) score+exp pair emission, fed just-in-time
    # as kT blocks complete so PE FIFO order never head-of-line blocks ACT
    es = {g: [] for g in range(NGROUPS)}
    # group 0 leads group 1 by LEAD chunks so the very first exps never
    # head-of-line block on q1's projection (which follows the first kacc)
    LEAD = 1
    SCHED = [(0, c) for c in range(LEAD)]
    for c in range(NKCH):
        if c + LEAD < NKCH:
            SCHED.append((0, c + LEAD))
        SCHED.append((1, c))
    _cursor = [0]  # position in SCHED (2 entries per chunk overall)

    def emit_one(g, ch):
        q0 = g * GW
        s_ps = pp_s.tile([P, 2 * GW], F32, tag="s", name="s_ps")
        nc.tensor.matmul(
            s_ps[:, 0:GW],
            kT_sb[0:H2, ch * P : (ch + 1) * P],
            qT_sb[0:H2, q0 : q0 + GW],
        )
        nc.tensor.matmul(
            s_ps[:, GW : 2 * GW],
            kT_sb[H2:H, ch * P : (ch + 1) * P],
            qT_sb[H2:H, q0 : q0 + GW],
        )
        e_sb = epool.tile([P, 2 * GW], BF16, tag="e", name="e_sb")
        nc.scalar.activation(e_sb, s_ps, AF.Exp, scale=SCALE)
        es[g].append(e_sb)

    def emit_pairs(limit_ch, maxn):
        # maxn counts chunk-equivalents (2 SCHED entries)
        n = 2 * maxn
        while _cursor[0] < len(SCHED) and n > 0:
            g, ch = SCHED[_cursor[0]]
            if ch >= limit_ch:
                break
            emit_one(g, ch)
            _cursor[0] += 1
            n -= 1

    # ---- projections (x streamed as half-block tiles for tight deps) ----
    # DMA order: x0h0, wv, x0h1, wk, wq, x1h0, x1h1, x2... so the V->collective
    # gate (x0+wv) and the first-kT gate (wk+x1h0) resolve as early as possible.
    with tc.tile_pool(name="xstream", bufs=3) as xpool, \
         tc.tile_pool(name="pp_proj", space="PSUM", bufs=1) as pp_proj:

        def load_xh(pos, sl, quarters=False):
            xh = xpool.tile([P, DCH, 512], BF16, tag="x", name=f"x{pos}_{sl}")
            if quarters:
                for qq in range(4):
                    nc.sync.dma_start(
                        out=xh[:, qq * 4 : (qq + 1) * 4, :],
                        in_=xr[pos, :, qq * 4 : (qq + 1) * 4,
                               sl * 512 : (sl + 1) * 512],
                    )
            else:
                nc.sync.dma_start(
                    out=xh, in_=xr[pos, :, :, sl * 512 : (sl + 1) * 512]
                )
            return xh

        nc.sync.dma_start(out=wv_sb, in_=wvT)
        x0h = [load_xh(0, 0, quarters=True)]
        nc.sync.dma_start(out=wq_sb, in_=wqT)
        x0h.append(load_xh(0, 1, quarters=True))
        nc.sync.dma_start(out=wk_sb, in_=wkT)

        # PE p-state warm-up: the tensor engine only reaches 2.4GHz after
        # 3us of CONTINUOUS execution, and any idle gap resets the ramp.
        # Run discarded matmuls from t~0 so the V projection (which gates
        # the collective) starts at full clock with the ramp already paid.
        dummy_sb = consts.tile([P, 512], BF16)
        nc.vector.memset(dummy_sb, 0.0)

        def warm(n):
            s_dummy = pp_s.tile([P, 2 * GW], F32, tag="s", name="warm")
            for _ in range(n):
                nc.tensor.matmul(s_dummy[0:1, 0:512], ones_bf, dummy_sb)


        # V (own block): stationary x chunk [d,128s], moving wv [d,128h].
        # Both V halves back-to-back: the AllGather dispatch is the kernel's
        # longest gate (collective ~41us + B-phase ~28us behind it).
        def proj_q(hf2):
            qacc = pp_proj.tile([P, 512], F32, tag="kacc", bufs=2)
            for c in range(DCH):
                nc.tensor.matmul(
                    qacc, wq_sb[:, c, :], x0h[hf2][:, c, :],
                    start=(c == 0), stop=(c == DCH - 1),
                )
            nc.vector.tensor_copy(qT_sb[:, hf2 * 512 : (hf2 + 1) * 512], qacc)

        for hf in range(2):
            vacc = pp_proj.tile([P, 4, P], F32, tag="vacc", bufs=1)
            for j4 in range(4):
                for c in range(DCH):
                    nc.tensor.matmul(
                        vacc[:, j4, :],
                        x0h[hf][:, c, j4 * P : (j4 + 1) * P],
                        wv_sb[:, c, :],
                        start=(c == 0),
                        stop=(c == DCH - 1),
                    )
            nc.vector.tensor_copy(vpart_sb[:, hf * 4 : (hf + 1) * 4, :], vacc)
            if hf == 0:
                proj_q(0)  # fills the PE gap while x0h1 streams in
        nc.sync.dma_start(out=part_d, in_=vpart_sb.rearrange("p j h -> p (j h)"))
        nc.gpsimd.collective_compute(
            "AllGather",
            OP.bypass,
            replica_groups=[[0, 1, 2, 3], [4, 5, 6, 7]],
            ins=[part_d.opt()],
            outs=[full_d.opt()],
        )


        # kT for batch blocks 0..3 (stream positions 1..4) with score/exp
        # pairs interleaved as coverage grows
        for blk in range(NBLK):
            for sl in range(2):
                xh = load_xh(blk + 1, sl, quarters=(blk == 0))
                kacc = pp_proj.tile([P, 512], F32, tag="kacc", bufs=2)
                for c in range(DCH):
                    nc.tensor.matmul(
                        kacc, wk_sb[:, c, :], xh[:, c, :],
                        start=(c == 0), stop=(c == DCH - 1),
                    )
                nc.vector.tensor_copy(
                    kT_sb[:, blk * BLKW + sl * 512 : blk * BLKW + (sl + 1) * 512],
                    kacc,
                )
                if blk == 0 and sl == 0:
                    proj_q(1)
                emit_pairs(blk * 8 + (sl + 1) * 4, 5)

    # gathered V -> SBUF (batch order: host maps rank r -> block order)
    for r in range(NBLK):
        nc.sync.dma_start(
            out=v_sb[:, r * 8 : (r + 1) * 8, :],
            in_=full_d[r].rearrange("p (j h) -> p j h", j=8),
        )

    pp_u = ctx.enter_context(tc.tile_pool(name="pp_u", space="PSUM", bufs=1))
    pp_sum = ctx.enter_context(tc.tile_pool(name="pp_sum", space="PSUM", bufs=1))
    pp_u1h = ctx.enter_context(tc.tile_pool(name="pp_u1h", space="PSUM", bufs=1))

    # ---- attention ----
    # Phase A (both groups back-to-back so ACT never starves): scores + exp
    # for all 64 chunk-visits; e is buffered in SBUF.
    # Phase B (per group, after the V gather lands): PV + sums bursts via
    # e-stationary matmuls (sums outputs have free-size 1 => ~free).
    c_ = 1.0 - LAMBDA_INIT
    a_ = 1.0 / (H * c_ * c_)
    b_ = RMS_EPS / (c_ * c_)
    bias_b = consts.tile([P, 1], F32)
    nc.vector.memset(bias_b, b_)

    def bcast_h(t, j0):
        """[P,1] columns j0..j0+NJ of t broadcast along h: [P, NJ, P]"""
        base = t.opt()
        return bass.AP(
            tensor=t.tensor,
            offset=base.offset + j0,
            ap=[list(base.ap[0]), [1, NJ], [0, P]],
        )

    def u_of(g, u_ap, hh, j):
        if u_ap is not None:
            return u_ap[:, hh * NJ + j, :]
        return get_u(1, hh)[:, j, :]

    def post_group(g, u_ap, sums_ps):
        """normalize + combine + RMS + store for one group (u_ap may be
        PSUM or SBUF). Batched across the NJ q-subtiles."""
        r_sb = small.tile([P, 2 * NJ], F32, tag="r", bufs=2)
        nc.vector.reciprocal(r_sb, sums_ps)
        nc.vector.tensor_scalar_mul(r_sb[:, NJ:], r_sb[:, NJ:], lam)
        # attn = u1 * r1 - u2 * r2  (r broadcast along h via stride-0 AP)
        t2 = small.tile([P, NJ, P], F32, tag="t2", bufs=2)
        for j in range(NJ):
            nc.vector.tensor_tensor(
                t2[:, j, :], u_of(g, u_ap, 1, j),
                bass.AP(tensor=r_sb.tensor,
                        offset=r_sb.opt().offset + NJ + j,
                        ap=[list(r_sb.opt().ap[0]), [0, P]]),
                op=OP.mult,
            )
        t1 = small.tile([P, NJ, P], F32, tag="t1", bufs=2)
        for j in range(NJ):
            nc.vector.tensor_tensor(
                t1[:, j, :], u_of(g, u_ap, 0, j),
                bass.AP(tensor=r_sb.tensor,
                        offset=r_sb.opt().offset + j,
                        ap=[list(r_sb.opt().ap[0]), [0, P]]),
                op=OP.mult,
            )
        attn_sb = outp.tile([P, NJ, P], F32, tag="attn", bufs=2)
        nc.vector.tensor_tensor(attn_sb, t1, t2, op=OP.subtract)
        # RMS stats: per-j accumulated square sums into one [P, NJ] tile
        ssq = small.tile([P, NJ], F32, tag="ssq", bufs=2)
        sq_scr = small.tile([P, NJ, P], F32, tag="sqscr", bufs=2)
        for j in range(NJ):
            nc.scalar.activation(
                sq_scr[:, j, :], attn_sb[:, j, :], AF.Square,
                accum_out=ssq[:, j : j + 1],
            )
        root = small.tile([P, NJ], F32, tag="root", bufs=2)
        nc.scalar.activation(root, ssq, AF.Sqrt, scale=a_, bias=bias_b)
        rrms = small.tile([P, NJ], F32, tag="rrms", bufs=2)
        nc.vector.reciprocal(rrms, root)
        # batched store: o[p, j, h] = attn * rrms (bcast h) * rmsw (bcast j),
        # one wide DVE chain and ONE output DMA per group
        o_sb = outp.tile([P, NJ, H], F32, tag="o", bufs=1)
        nc.vector.tensor_tensor(o_sb, attn_sb, bcast_h(rrms, 0), op=OP.mult)
        rmsw4 = bass.AP(
            tensor=rmsw_bc.tensor,
            offset=rmsw_bc.opt().offset,
            ap=[list(rmsw_bc.opt().ap[0]), [0, NJ], [1, H]],
        )
        nc.vector.tensor_tensor(o_sb, o_sb, rmsw4, op=OP.mult)
        nc.sync.dma_start(
            out=bass.AP(
                tensor=out_d.tensor,
                offset=g * GW * H,
                ap=[[H, P], [P * H, NJ], [1, H]],
            ),
            in_=o_sb,
        )

    # Phase B. ch-outer / hj-inner with ONE accumulation group per 2KB psum
    # zero region: u bank0 = cells hj 0..3, u bank1 = hj 4..7, sums = all 8
    # cells in one bank. start fires on the region's first write, stop on
    # its last. B(g0) chunks are interleaved into the late score pairs (see
    # emit loop below) so they don't serialize behind the score tail.
    # PSUM bank plan after proj releases (4 free banks): u0 (2 banks) +
    # shared sums (1 bank, all 16 cells, ONE accumulation group spanning the
    # whole B phase) + u1-half1 (1 bank) accumulate CONCURRENTLY; u1-half2
    # reuses u0's slot after the copy-out. This halves the serialized B1
    # tail that used to run entirely after B0 closed.
    b_tiles = {}
    scount = [0]
    SUMS_TOTAL = 2 * NKCH * 2 * NJ

    def get_sums():
        if "s" not in b_tiles:
            b_tiles["s"] = pp_sum.tile([P, 4 * NJ], F32, tag="sum",
                                       name="sums", bufs=1)
        return b_tiles["s"]

    def get_u(g, hh):
        # u0: one [P, 8, P] tile (2 banks, both halves). u1h1: its own
        # 1-bank tile. u1h2: a fresh tag-u tile (waits for u0's release).
        key = (g, hh)
        if key not in b_tiles:
            if g == 0:
                t = b_tiles.get((0, 0))
                if t is None:
                    t = pp_u.tile([P, 2 * NJ, P], F32, tag="u", name="u0")
                b_tiles[(0, 0)] = t
                b_tiles[(0, 1)] = t
            elif hh == 0:
                b_tiles[key] = pp_u1h.tile([P, NJ, P], F32, tag="u1h",
                                           name="u1h1", bufs=1)
            else:
                b_tiles[key] = pp_u.tile([P, 2 * NJ, P], F32, tag="u",
                                         name="u1h2")
        return b_tiles[(g, hh)]

    def emit_B(g, hh, ch):
        u_ps = get_u(g, hh)
        sums_ps = get_sums()
        for j in range(NJ):
            hj = hh * NJ + j
            e_sub = es[g][ch][:, hj * P : (hj + 1) * P]
            if g == 0:
                u_slice = u_ps[:, hj, :]
            else:
                u_slice = u_ps[:, j, :]
            nc.tensor.matmul(
                u_slice, e_sub, v_sb[:, ch, :],
                start=(ch == 0 and j == 0),
                stop=(ch == NKCH - 1 and j == NJ - 1),
                skip_group_check=True,
            )
            nc.tensor.matmul(
                sums_ps[:, g * 2 * NJ + hj : g * 2 * NJ + hj + 1],
                e_sub, ones_bf,
                start=(scount[0] == 0),
                stop=(scount[0] == SUMS_TOTAL - 1),
                skip_group_check=True,
            )
            scount[0] += 1

    # three B work queues: q0 = B0 chunks (both halves, 2 banks), q1 = B1
    # half1 (own bank, CONCURRENT with B0), q2 = B1 half2 (after u0 frees).
    bcur = [0, 0, 0]
    g0_closed = [False]

    def close_g0():
        u_sb = usb_pool.tile([P, 2 * NJ, P], F32, tag="usb")
        nc.vector.tensor_copy(u_sb, get_u(0, 0))
        post_group(0, u_sb, get_sums()[:, 0 : 2 * NJ])
        g0_closed[0] = True

    def emit_B0(ch):
        emit_B(0, 0, ch)
        emit_B(0, 1, ch)

    def maybe_B(n):
        while n > 0 and _cursor[0] >= 38:
            did = False
            if bcur[0] < NKCH and bcur[0] < len(es[0]):
                emit_B0(bcur[0])
                bcur[0] += 1
                n -= 1
                did = True
            if n > 0 and bcur[1] < NKCH and bcur[1] < len(es[1]):
                emit_B(1, 0, bcur[1])
                bcur[1] += 1
                n -= 1
                did = True
            if (n > 0 and bcur[0] >= NKCH and bcur[2] < NKCH
                    and bcur[2] < len(es[1])):
                if not g0_closed[0]:
                    close_g0()
                emit_B(1, 1, bcur[2])
                bcur[2] += 1
                n -= 1
                did = True
            if not did:
                break

    while _cursor[0] < len(SCHED):
        emit_pairs(NKCH, 1)
        maybe_B(4)
    while bcur[0] < NKCH:
        emit_B0(bcur[0])
        bcur[0] += 1
    while bcur[1] < NKCH:
        emit_B(1, 0, bcur[1])
        bcur[1] += 1
    if not g0_closed[0]:
        close_g0()
    while bcur[2] < NKCH:
        emit_B(1, 1, bcur[2])
        bcur[2] += 1
    post_group(1, None, get_sums()[:, 2 * NJ : 4 * NJ])


def build(lam: float):
    from concourse._compat import axon_active

    nc = bacc.Bacc(
        "TRN2",
        target_bir_lowering=False,
        debug=not axon_active(),
        num_devices=NCORES,
    )
    with tile.TileContext(nc) as tc:
        with ExitStack() as ctx:
            _emit(ctx, tc, lam)
    nc.compile()
    return nc


def make_in_maps(x, Wq, Wk, Wv, rms_weight):
    bf = ml_dtypes.bfloat16
    x = np.asarray(x, dtype=np.float32)
    xT = np.ascontiguousarray(x.transpose(0, 2, 1)).astype(bf)  # [B, D, S]
    def warr(W):
        # [P, DCH, H]: warr[p, c, h] = W.T[c*128 + p, h]
        wT = np.asarray(W, np.float32).T.reshape(DCH, P, H)
        return np.ascontiguousarray(wT.transpose(1, 0, 2)).astype(bf)

    wqT, wkT, wvT = warr(Wq), warr(Wk), warr(Wv)
    rw = np.ascontiguousarray(np.asarray(rms_weight, np.float32))
    in_maps = []
    for core in range(NCORES):
        b, qb = divmod(core, NCORES // B)
        # 5 xT blocks: position 0 = own block (feeds V/q projections),
        # positions 1..4 = batch blocks 0..3 (feed kT in batch order, which
        # matches the V AllGather rank order). The duplicate own block keeps
        # the SPMD program free of per-core block-index knowledge.
        blocks = [xT[b][:, qb * BLKW : (qb + 1) * BLKW]]
        for r in range(NBLK):
            blocks.append(xT[b][:, r * BLKW : (r + 1) * BLKW])
        in_maps.append(
            {
                "xT": np.ascontiguousarray(np.stack(blocks)),
                "wqT": wqT,
                "wkT": wkT,
                "wvT": wvT,
                "rmsw": rw,
            }
        )
    return in_maps


def kernel(x, Wq, Wk, Wv, lambda_q1, lambda_q2, lambda_k1, lambda_k2, rms_weight):
    lq1 = np.asarray(lambda_q1, np.float32)
    lq2 = np.asarray(lambda_q2, np.float32)
    lk1 = np.asarray(lambda_k1, np.float32)
    lk2 = np.asarray(lambda_k2, np.float32)
    lam = float(np.exp(np.dot(lq1, lk1)) - np.exp(np.dot(lq2, lk2)) + LAMBDA_INIT)
    nc = build(lam)
    in_maps = make_in_maps(x, Wq, Wk, Wv, rms_weight)
    res = bass_utils.run_bass_kernel_spmd(nc, in_maps, core_ids=list(range(NCORES)))
    out = np.empty((B, S, H), np.float32)
    for core in range(NCORES):
        b, qb = divmod(core, NCORES // B)
        out[b, qb * QSHARD : (qb + 1) * QSHARD] = res.results[core]["out"]
    return out


# revision 6
# speedup vs baseline: 1.0084x; 1.0062x over previous
"""DiffAttn Trainium2 Bass kernel, v2: replicate-K / gather-V design.

Each core handles (batch b, q-block qb) = divmod(core, 4): 1024 q rows.
It streams the FULL batch xT (16MB) and projects kT for ALL 4096 keys
locally (scores never wait on a collective), projects V for its OWN
1024-row block, and AllGathers the V parts (1MB out, ~41us modeled,
hidden under scores/exp).

Layout: scores TRANSPOSED ([sk, q], keys on partitions) so exp(scores)
feeds PV directly with contraction over sk. PV uses e-subtiles as the
STATIONARY operand (out [q,128pt x h] natural orientation, cost = moving
h-cols = same as before, but U needs no post transpose) and row-sums
ride the same stationaries with a ones moving vector (out free-size 1 =
~free in the cost model, killing the old ones-matmul pass entirely).

PV/sums for ALL blocks are deferred until the V gather lands (~49us);
e tiles are buffered in SBUF meanwhile (group 0 worst case ~35 tiles).
"""

import math
import os
import sys
from contextlib import ExitStack

import numpy as np

for _p in ("/root/.axon_site/_ro/trn_rl_repo", "/opt/trn_rl_repo"):
    if os.path.isdir(_p) and _p not in sys.path:
        sys.path.append(_p)

import ml_dtypes  # noqa: E402

import concourse.bass as bass  # noqa: E402
import concourse.mybir as mybir  # noqa: E402
import concourse.tile as tile  # noqa: E402
from concourse import bacc, bass_utils  # noqa: E402

B, S, D, H = 2, 4096, 2048, 128
H2 = H // 2  # 64
P = 128
NCORES = 8
QSHARD = 1024  # q rows per core
DCH = D // P  # 16 d-chunks
NKCH = S // P  # 32 key chunks of 128
NBLK, BLKW = 4, 1024  # key blocks
NGROUPS, GW = 2, 512  # q groups per core
NJ = GW // P  # 4 q sub-blocks of 128 per group

LAMBDA_INIT = 0.8 - 0.6 * math.exp(-0.3 * 12)
RMS_EPS = float(np.finfo(np.float32).eps)
SCALE = 1.0 / math.sqrt(H2)

F32 = mybir.dt.float32
BF16 = mybir.dt.bfloat16

AF = mybir.ActivationFunctionType
OP = mybir.AluOpType


def _emit(ctx: ExitStack, tc: "tile.TileContext", lam: float):
    nc = tc.nc

    # xT: 5 blocks of [D, 1024] bf16. Position 0 = OWN block (feeds V and q
    # projections); positions 1..4 = the batch's blocks 0..3 in batch order
    # (feed kT, whose chunk index must match the V AllGather rank order).
    # The own block is duplicated (one extra hidden 4MB DMA) so the SPMD
    # program needs no per-core knowledge of its block index.
    xT = nc.dram_tensor("xT", (NBLK + 1, D, BLKW), BF16, kind="ExternalInput").ap()
    # weights pre-arranged on host as [p, c, h] so the DMA rows are 4KB
    # contiguous (sub-512B elements pay a 2x DMA latency penalty)
    wqT = nc.dram_tensor("wqT", (P, DCH, H), BF16, kind="ExternalInput").ap()
    wkT = nc.dram_tensor("wkT", (P, DCH, H), BF16, kind="ExternalInput").ap()
    wvT = nc.dram_tensor("wvT", (P, DCH, H), BF16, kind="ExternalInput").ap()
    rmsw = nc.dram_tensor("rmsw", (H,), F32, kind="ExternalInput").ap()
    part_d = nc.dram_tensor("part_d", (P, BLKW), BF16).ap()
    full_d = nc.dram_tensor("full_d", (NBLK, P, BLKW), BF16).ap()
    out_d = nc.dram_tensor("out", (QSHARD, H), F32, kind="ExternalOutput").ap()

    consts = ctx.enter_context(tc.tile_pool(name="consts", bufs=1))
    persist = ctx.enter_context(tc.tile_pool(name="persist", bufs=1))

    ones_bf = consts.tile([P, 1], BF16)
    nc.vector.memset(ones_bf, 1.0)
    rmsw_bc = consts.tile([P, H], F32)
    nc.sync.dma_start(
        out=rmsw_bc,
        in_=bass.AP(tensor=rmsw.tensor, offset=0, ap=[[0, P], [1, H]]),
    )
    wq_sb = consts.tile([P, DCH, H], BF16)
    wk_sb = consts.tile([P, DCH, H], BF16)
    wv_sb = consts.tile([P, DCH, H], BF16)

    kT_sb = persist.tile([P, S], BF16)  # [h, sk] all 4 blocks, batch order
    qT_sb = persist.tile([P, QSHARD], BF16)  # [h, sq]
    vpart_sb = persist.tile([P, 8, P], BF16)  # own block V [sk%128, ch, h]
    v_sb = persist.tile([P, NKCH, P], BF16)  # gathered V, batch order

    xr = xT.rearrange("b (c p) w -> b p c w", p=P)
    epool = ctx.enter_context(tc.tile_pool(name="epool", bufs=52))
    small = ctx.enter_context(tc.tile_pool(name="small", bufs=8))
    outp = ctx.enter_context(tc.tile_pool(name="outp", bufs=4))
    usb_pool = ctx.enter_context(tc.tile_pool(name="usb", bufs=1))

    # scores PSUM pool first so it does NOT share banks with (and thus wait
    # on) the projection pool: 4 + 3 banks coexist, u/sum pools reuse the
    # proj banks after release (they wait for the V gather anyway).
    pp_s = ctx.enter_context(tc.tile_pool(name="pp_s", space="PSUM", bufs=2))

    # interleaved (g0, ch)(g1, ch) score+exp pair emission, fed just-in-time
    # as kT blocks complete so PE FIFO order never head-of-line blocks ACT
    es = {g: [] for g in range(NGROUPS)}
    # group 0 leads group 1 by LEAD chunks so the very first exps never
    # head-of-line block on q1's projection (which follows the first kacc)
    LEAD = 1
    SCHED = [(0, c) for c in range(LEAD)]
    for c in range(NKCH):
        if c + LEAD < NKCH:
            SCHED.append((0, c + LEAD))
        SCHED.append((1, c))
    _cursor = [0]  # position in SCHED (2 entries per chunk overall)

    def emit_one(g, ch):
        q0 = g * GW
        s_ps = pp_s.tile([P, 2 * GW], F32, tag="s", name="s_ps")
        nc.tensor.matmul(
            s_ps[:, 0:GW],
            kT_sb[0:H2, ch * P : (ch + 1) * P],
            qT_sb[0:H2, q0 : q0 + GW],
        )
        nc.tensor.matmul(
            s_ps[:, GW : 2 * GW],
            kT_sb[H2:H, ch * P : (ch + 1) * P],
            qT_sb[H2:H, q0 : q0 + GW],
        )
        e_sb = epool.tile([P, 2 * GW], BF16, tag="e", name="e_sb")
        nc.scalar.activation(e_sb, s_ps, AF.Exp, scale=SCALE)
        es[g].append(e_sb)

    def emit_pairs(limit_ch, maxn):
        # maxn counts chunk-equivalents (2 SCHED entries)
        n = 2 * maxn
        while _cursor[0] < len(SCHED) and n > 0:
            g, ch = SCHED[_cursor[0]]
            if ch >= limit_ch:
                break
            emit_one(g, ch)
            _cursor[0] += 1
            n -= 1

    # ---- projections (x streamed as half-block tiles for tight deps) ----
    # DMA order: x0h0, wv, x0h1, wk, wq, x1h0, x1h1, x2... so the V->collective
    # gate (x0+wv) and the first-kT gate (wk+x1h0) resolve as early as possible.
    with tc.tile_pool(name="xstream", bufs=3) as xpool, \
         tc.tile_pool(name="pp_proj", space="PSUM", bufs=1) as pp_proj:

        def load_xh(pos, sl, quarters=False):
            xh = xpool.tile([P, DCH, 512], BF16, tag="x", name=f"x{pos}_{sl}")
            if quarters:
                for qq in range(4):
                    nc.sync.dma_start(
                        out=xh[:, qq * 4 : (qq + 1) * 4, :],
                        in_=xr[pos, :, qq * 4 : (qq + 1) * 4,
                               sl * 512 : (sl + 1) * 512],
                    )
            else:
                nc.sync.dma_start(
                    out=xh, in_=xr[pos, :, :, sl * 512 : (sl + 1) * 512]
                )
            return xh

        nc.sync.dma_start(out=wv_sb, in_=wvT)
        x0h = [load_xh(0, 0, quarters=True)]
        nc.sync.dma_start(out=wq_sb, in_=wqT)
        x0h.append(load_xh(0, 1, quarters=True))
        nc.sync.dma_start(out=wk_sb, in_=wkT)

        # PE p-state warm-up: the tensor engine only reaches 2.4GHz after
        # 3us of CONTINUOUS execution, and any idle gap resets the ramp.
        # Run discarded matmuls from t~0 so the V projection (which gates
        # the collective) starts at full clock with the ramp already paid.
        dummy_sb = consts.tile([P, 512], BF16)
        nc.vector.memset(dummy_sb, 0.0)

        def warm(n):
            s_dummy = pp_s.tile([P, 2 * GW], F32, tag="s", name="warm")
            for _ in range(n):
                nc.tensor.matmul(s_dummy[0:1, 0:512], ones_bf, dummy_sb)


        # V (own block): stationary x chunk [d,128s], moving wv [d,128h].
        # Both V halves back-to-back: the AllGather dispatch is the kernel's
        # longest gate (collective ~41us + B-phase ~28us behind it).
        def proj_q(hf2):
            qacc = pp_proj.tile([P, 512], F32, tag="kacc", bufs=2)
            for c in range(DCH):
                nc.tensor.matmul(
                    qacc, wq_sb[:, c, :], x0h[hf2][:, c, :],
                    start=(c == 0), stop=(c == DCH - 1),
                )
            nc.vector.tensor_copy(qT_sb[:, hf2 * 512 : (hf2 + 1) * 512], qacc)

        for hf in range(2):
            vacc = pp_proj.tile([P, 4, P], F32, tag="vacc", bufs=1)
            for j4 in range(4):
                for c in range(DCH):
                    nc.tensor.matmul(
                        vacc[:, j4, :],
                        x0h[hf][:, c, j4 * P : (j4 + 1) * P],
                        wv_sb[:, c, :],
                        start=(c == 0),
                        stop=(c == DCH - 1),
                    )
            nc.vector.tensor_copy(vpart_sb[:, hf * 4 : (hf + 1) * 4, :], vacc)
            if hf == 0:
                proj_q(0)  # fills the PE gap while x0h1 streams in
        nc.sync.dma_start(out=part_d, in_=vpart_sb.rearrange("p j h -> p (j h)"))
        nc.gpsimd.collective_compute(
            "AllGather",
            OP.bypass,
            replica_groups=[[0, 1, 2, 3], [4, 5, 6, 7]],
            ins=[part_d.opt()],
            outs=[full_d.opt()],
        )


        # kT for batch blocks 0..3 (stream positions 1..4) with score/exp
        # pairs interleaved as coverage grows
        for blk in range(NBLK):
            for sl in range(2):
                xh = load_xh(blk + 1, sl, quarters=(blk == 0))
                kacc = pp_proj.tile([P, 512], F32, tag="kacc", bufs=2)
                for c in range(DCH):
                    nc.tensor.matmul(
                        kacc, wk_sb[:, c, :], xh[:, c, :],
                        start=(c == 0), stop=(c == DCH - 1),
                    )
                nc.vector.tensor_copy(
                    kT_sb[:, blk * BLKW + sl * 512 : blk * BLKW + (sl + 1) * 512],
                    kacc,
                )
                if blk == 0 and sl == 0:
                    proj_q(1)
                emit_pairs(blk * 8 + (sl + 1) * 4, 5)

    # gathered V -> SBUF (batch order: host maps rank r -> block order)
    for r in range(NBLK):
        nc.sync.dma_start(
            out=v_sb[:, r * 8 : (r + 1) * 8, :],
            in_=full_d[r].rearrange("p (j h) -> p j h", j=8),
        )

    pp_u = ctx.enter_context(tc.tile_pool(name="pp_u", space="PSUM", bufs=1))
    pp_sum = ctx.enter_context(tc.tile_pool(name="pp_sum", space="PSUM", bufs=1))
    pp_u1h = ctx.enter_context(tc.tile_pool(name="pp_u1h", space="PSUM", bufs=1))

    # ---- attention ----
    # Phase A (both groups back-to-back so ACT never starves): scores + exp
    # for all 64 chunk-visits; e is buffered in SBUF.
    # Phase B (per group, after the V gather lands): PV + sums bursts via
    # e-stationary matmuls (sums outputs have free-size 1 => ~free).
    c_ = 1.0 - LAMBDA_INIT
    a_ = 1.0 / (H * c_ * c_)
    b_ = RMS_EPS / (c_ * c_)
    bias_b = consts.tile([P, 1], F32)
    nc.vector.memset(bias_b, b_)

    def bcast_h(t, j0):
        """[P,1] columns j0..j0+NJ of t broadcast along h: [P, NJ, P]"""
        base = t.opt()
        return bass.AP(
            tensor=t.tensor,
            offset=base.offset + j0,
            ap=[list(base.ap[0]), [1, NJ], [0, P]],
        )

    def u_of(g, u_ap, hh, j):
        if u_ap is not None:
            return u_ap[:, hh * NJ + j, :]
        return get_u(1, hh)[:, j, :]

    def post_group(g, u_ap, sums_ps):
        """normalize + combine + RMS + store for one group (u_ap may be
        PSUM or SBUF). Batched across the NJ q-subtiles."""
        r_sb = small.tile([P, 2 * NJ], F32, tag="r", bufs=2)
        nc.vector.reciprocal(r_sb, sums_ps)
        nc.vector.tensor_scalar_mul(r_sb[:, NJ:], r_sb[:, NJ:], lam)
        # attn = u1 * r1 - u2 * r2  (r broadcast along h via stride-0 AP)
        t2 = small.tile([P, NJ, P], F32, tag="t2", bufs=2)
        for j in range(NJ):
            nc.vector.tensor_tensor(
                t2[:, j, :], u_of(g, u_ap, 1, j),
                bass.AP(tensor=r_sb.tensor,
                        offset=r_sb.opt().offset + NJ + j,
                        ap=[list(r_sb.opt().ap[0]), [0, P]]),
                op=OP.mult,
            )
        t1 = small.tile([P, NJ, P], F32, tag="t1", bufs=2)
        for j in range(NJ):
            nc.vector.tensor_tensor(
                t1[:, j, :], u_of(g, u_ap, 0, j),
                bass.AP(tensor=r_sb.tensor,
                        offset=r_sb.opt().offset + j,
                        ap=[list(r_sb.opt().ap[0]), [0, P]]),
                op=OP.mult,
            )
        attn_sb = outp.tile([P, NJ, P], F32, tag="attn", bufs=2)
        nc.vector.tensor_tensor(attn_sb, t1, t2, op=OP.subtract)
        # RMS stats: per-j accumulated square sums into one [P, NJ] tile
        ssq = small.tile([P, NJ], F32, tag="ssq", bufs=2)
        sq_scr = small.tile([P, NJ, P], F32, tag="sqscr", bufs=2)
        for j in range(NJ):
            nc.scalar.activation(
                sq_scr[:, j, :], attn_sb[:, j, :], AF.Square,
                accum_out=ssq[:, j : j + 1],
            )
        root = small.tile([P, NJ], F32, tag="root", bufs=2)
        nc.scalar.activation(root, ssq, AF.Sqrt, scale=a_, bias=bias_b)
        rrms = small.tile([P, NJ], F32, tag="rrms", bufs=2)
        nc.vector.reciprocal(rrms, root)
        # batched store: o[p, j, h] = attn * rrms (bcast h) * rmsw (bcast j),
        # one wide DVE chain and ONE output DMA per group
        o_sb = outp.tile([P, NJ, H], F32, tag="o", bufs=1)
        for j in range(NJ):
            nc.vector.scalar_tensor_tensor(
                o_sb[:, j, :], attn_sb[:, j, :], rrms[:, j : j + 1], rmsw_bc,
                op0=OP.mult, op1=OP.mult,
            )
        nc.sync.dma_start(
            out=bass.AP(
                tensor=out_d.tensor,
                offset=g * GW * H,
                ap=[[H, P], [P * H, NJ], [1, H]],
            ),
            in_=o_sb,
        )

    # Phase B. ch-outer / hj-inner with ONE accumulation group per 2KB psum
    # zero region: u bank0 = cells hj 0..3, u bank1 = hj 4..7, sums = all 8
    # cells in one bank. start fires on the region's first write, stop on
    # its last. B(g0) chunks are interleaved into the late score pairs (see
    # emit loop below) so they don't serialize behind the score tail.
    # PSUM bank plan after proj releases (4 free banks): u0 (2 banks) +
    # shared sums (1 bank, all 16 cells, ONE accumulation group spanning the
    # whole B phase) + u1-half1 (1 bank) accumulate CONCURRENTLY; u1-half2
    # reuses u0's slot after the copy-out. This halves the serialized B1
    # tail that used to run entirely after B0 closed.
    b_tiles = {}
    scount = [0]
    SUMS_TOTAL = 2 * NKCH * 2 * NJ

    def get_sums():
        if "s" not in b_tiles:
            b_tiles["s"] = pp_sum.tile([P, 4 * NJ], F32, tag="sum",
                                       name="sums", bufs=1)
        return b_tiles["s"]

    def get_u(g, hh):
        # u0: one [P, 8, P] tile (2 banks, both halves). u1h1: its own
        # 1-bank tile. u1h2: a fresh tag-u tile (waits for u0's release).
        key = (g, hh)
        if key not in b_tiles:
            if g == 0:
                t = b_tiles.get((0, 0))
                if t is None:
                    t = pp_u.tile([P, 2 * NJ, P], F32, tag="u", name="u0")
                b_tiles[(0, 0)] = t
                b_tiles[(0, 1)] = t
            elif hh == 0:
                b_tiles[key] = pp_u1h.tile([P, NJ, P], F32, tag="u1h",
                                           name="u1h1", bufs=1)
            else:
                b_tiles[key] = pp_u.tile([P, 2 * NJ, P], F32, tag="u",
                                         name="u1h2")
        return b_tiles[(g, hh)]

    def emit_B(g, hh, ch):
        u_ps = get_u(g, hh)
        sums_ps = get_sums()
        for j in range(NJ):
            hj = hh * NJ + j
            e_sub = es[g][ch][:, hj * P : (hj + 1) * P]
            if g == 0:
                u_slice = u_ps[:, hj, :]
            else:
                u_slice = u_ps[:, j, :]
            nc.tensor.matmul(
                u_slice, e_sub, v_sb[:, ch, :],
                start=(ch == 0 and j == 0),
                stop=(ch == NKCH - 1 and j == NJ - 1),
                skip_group_check=True,
            )
            nc.tensor.matmul(
                sums_ps[:, g * 2 * NJ + hj : g * 2 * NJ + hj + 1],
                e_sub, ones_bf,
                start=(scount[0] == 0),
                stop=(scount[0] == SUMS_TOTAL - 1),
                skip_group_check=True,
            )
            scount[0] += 1

    # three B work queues: q0 = B0 chunks (both halves, 2 banks), q1 = B1
    # half1 (own bank, CONCURRENT with B0), q2 = B1 half2 (after u0 frees).
    bcur = [0, 0, 0]
    g0_closed = [False]

    def close_g0():
        u_sb = usb_pool.tile([P, 2 * NJ, P], F32, tag="usb")
        nc.vector.tensor_copy(u_sb, get_u(0, 0))
        post_group(0, u_sb, get_sums()[:, 0 : 2 * NJ])
        g0_closed[0] = True

    def emit_B0(ch):
        emit_B(0, 0, ch)
        emit_B(0, 1, ch)

    def maybe_B(n):
        while n > 0 and _cursor[0] >= 38:
            did = False
            if bcur[0] < NKCH and bcur[0] < len(es[0]):
                emit_B0(bcur[0])
                bcur[0] += 1
                n -= 1
                did = True
            if n > 0 and bcur[1] < NKCH and bcur[1] < len(es[1]):
                emit_B(1, 0, bcur[1])
                bcur[1] += 1
                n -= 1
                did = True
            if (n > 0 and bcur[0] >= NKCH and bcur[2] < NKCH
                    and bcur[2] < len(es[1])):
                if not g0_closed[0]:
                    close_g0()
                emit_B(1, 1, bcur[2])
                bcur[2] += 1
                n -= 1
                did = True
            if not did:
                break

    while _cursor[0] < len(SCHED):
        emit_pairs(NKCH, 1)
        maybe_B(4)
    while bcur[0] < NKCH:
        emit_B0(bcur[0])
        bcur[0] += 1
    while bcur[1] < NKCH:
        emit_B(1, 0, bcur[1])
        bcur[1] += 1
    if not g0_closed[0]:
        close_g0()
    while bcur[2] < NKCH:
        emit_B(1, 1, bcur[2])
        bcur[2] += 1
    post_group(1, None, get_sums()[:, 2 * NJ : 4 * NJ])


def build(lam: float):
    from concourse._compat import axon_active

    nc = bacc.Bacc(
        "TRN2",
        target_bir_lowering=False,
        debug=not axon_active(),
        num_devices=NCORES,
    )
    with tile.TileContext(nc) as tc:
        with ExitStack() as ctx:
            _emit(ctx, tc, lam)
    nc.compile()
    return nc


def make_in_maps(x, Wq, Wk, Wv, rms_weight):
    bf = ml_dtypes.bfloat16
    x = np.asarray(x, dtype=np.float32)
    xT = np.ascontiguousarray(x.transpose(0, 2, 1)).astype(bf)  # [B, D, S]
    def warr(W):
        # [P, DCH, H]: warr[p, c, h] = W.T[c*128 + p, h]
        wT = np.asarray(W, np.float32).T.reshape(DCH, P, H)
        return np.ascontiguousarray(wT.transpose(1, 0, 2)).astype(bf)

    wqT, wkT, wvT = warr(Wq), warr(Wk), warr(Wv)
    rw = np.ascontiguousarray(np.asarray(rms_weight, np.float32))
    in_maps = []
    for core in range(NCORES):
        b, qb = divmod(core, NCORES // B)
        # 5 xT blocks: position 0 = own block (feeds V/q projections),
        # positions 1..4 = batch blocks 0..3 (feed kT in batch order, which
        # matches the V AllGather rank order). The duplicate own block keeps
        # the SPMD program free of per-core block-index knowledge.
        blocks = [xT[b][:, qb * BLKW : (qb + 1) * BLKW]]
        for r in range(NBLK):
            blocks.append(xT[b][:, r * BLKW : (r + 1) * BLKW])
        in_maps.append(
            {
                "xT": np.ascontiguousarray(np.stack(blocks)),
                "wqT": wqT,
                "wkT": wkT,
                "wvT": wvT,
                "rmsw": rw,
            }
        )
    return in_maps


def kernel(x, Wq, Wk, Wv, lambda_q1, lambda_q2, lambda_k1, lambda_k2, rms_weight):
    lq1 = np.asarray(lambda_q1, np.float32)
    lq2 = np.asarray(lambda_q2, np.float32)
    lk1 = np.asarray(lambda_k1, np.float32)
    lk2 = np.asarray(lambda_k2, np.float32)
    lam = float(np.exp(np.dot(lq1, lk1)) - np.exp(np.dot(lq2, lk2)) + LAMBDA_INIT)
    nc = build(lam)
    in_maps = make_in_maps(x, Wq, Wk, Wv, rms_weight)
    res = bass_utils.run_bass_kernel_spmd(nc, in_maps, core_ids=list(range(NCORES)))
    out = np.empty((B, S, H), np.float32)
    for core in range(NCORES):
        b, qb = divmod(core, NCORES // B)
        out[b, qb * QSHARD : (qb + 1) * QSHARD] = res.results[core]["out"]
    return out


# revision 7
# speedup vs baseline: 1.0224x; 1.0139x over previous
"""DiffAttn Trainium2 Bass kernel, v2: replicate-K / gather-V design.

Each core handles (batch b, q-block qb) = divmod(core, 4): 1024 q rows.
It streams the FULL batch xT (16MB) and projects kT for ALL 4096 keys
locally (scores never wait on a collective), projects V for its OWN
1024-row block, and AllGathers the V parts (1MB out, ~41us modeled,
hidden under scores/exp).

Layout: scores TRANSPOSED ([sk, q], keys on partitions) so exp(scores)
feeds PV directly with contraction over sk. PV uses e-subtiles as the
STATIONARY operand (out [q,128pt x h] natural orientation, cost = moving
h-cols = same as before, but U needs no post transpose) and row-sums
ride the same stationaries with a ones moving vector (out free-size 1 =
~free in the cost model, killing the old ones-matmul pass entirely).

PV/sums for ALL blocks are deferred until the V gather lands (~49us);
e tiles are buffered in SBUF meanwhile (group 0 worst case ~35 tiles).
"""

import math
import os
import sys
from contextlib import ExitStack

import numpy as np

for _p in ("/root/.axon_site/_ro/trn_rl_repo", "/opt/trn_rl_repo"):
    if os.path.isdir(_p) and _p not in sys.path:
        sys.path.append(_p)

import ml_dtypes  # noqa: E402

import concourse.bass as bass  # noqa: E402
import concourse.mybir as mybir  # noqa: E402
import concourse.tile as tile  # noqa: E402
from concourse import bacc, bass_utils  # noqa: E402

B, S, D, H = 2, 4096, 2048, 128
H2 = H // 2  # 64
P = 128
NCORES = 8
QSHARD = 1024  # q rows per core
DCH = D // P  # 16 d-chunks
NKCH = S // P  # 32 key chunks of 128
NBLK, BLKW = 4, 1024  # key blocks
NGROUPS, GW = 2, 512  # q groups per core
NJ = GW // P  # 4 q sub-blocks of 128 per group

LAMBDA_INIT = 0.8 - 0.6 * math.exp(-0.3 * 12)
RMS_EPS = float(np.finfo(np.float32).eps)
SCALE = 1.0 / math.sqrt(H2)

F32 = mybir.dt.float32
BF16 = mybir.dt.bfloat16

AF = mybir.ActivationFunctionType
OP = mybir.AluOpType


def _emit(ctx: ExitStack, tc: "tile.TileContext", lam: float):
    nc = tc.nc

    # xT: 5 blocks of [D, 1024] bf16. Position 0 = OWN block (feeds V and q
    # projections); positions 1..4 = the batch's blocks 0..3 in batch order
    # (feed kT, whose chunk index must match the V AllGather rank order).
    # The own block is duplicated (one extra hidden 4MB DMA) so the SPMD
    # program needs no per-core knowledge of its block index.
    xT = nc.dram_tensor("xT", (NBLK + 1, D, BLKW), BF16, kind="ExternalInput").ap()
    # weights pre-arranged on host as [p, c, h] so the DMA rows are 4KB
    # contiguous (sub-512B elements pay a 2x DMA latency penalty)
    wqT = nc.dram_tensor("wqT", (P, DCH, H), BF16, kind="ExternalInput").ap()
    wkT = nc.dram_tensor("wkT", (P, DCH, H), BF16, kind="ExternalInput").ap()
    wvT = nc.dram_tensor("wvT", (P, DCH, H), BF16, kind="ExternalInput").ap()
    rmsw = nc.dram_tensor("rmsw", (H,), F32, kind="ExternalInput").ap()
    part_d = nc.dram_tensor("part_d", (P, BLKW), BF16).ap()
    full_d = nc.dram_tensor("full_d", (NBLK, P, BLKW), BF16).ap()
    out_d = nc.dram_tensor("out", (QSHARD, H), F32, kind="ExternalOutput").ap()

    consts = ctx.enter_context(tc.tile_pool(name="consts", bufs=1))
    persist = ctx.enter_context(tc.tile_pool(name="persist", bufs=1))

    ones_bf = consts.tile([P, 1], BF16)
    nc.vector.memset(ones_bf, 1.0)
    rmsw_bc = consts.tile([P, H], F32)
    nc.sync.dma_start(
        out=rmsw_bc,
        in_=bass.AP(tensor=rmsw.tensor, offset=0, ap=[[0, P], [1, H]]),
    )
    wq_sb = consts.tile([P, DCH, H], BF16)
    wk_sb = consts.tile([P, DCH, H], BF16)
    wv_sb = consts.tile([P, DCH, H], BF16)

    kT_sb = persist.tile([P, S], BF16)  # [h, sk] all 4 blocks, batch order
    qT_sb = persist.tile([P, QSHARD], BF16)  # [h, sq]
    vpart_sb = persist.tile([P, 8, P], BF16)  # own block V [sk%128, ch, h]
    v_sb = persist.tile([P, NKCH, P], BF16)  # gathered V, batch order

    xr = xT.rearrange("b (c p) w -> b p c w", p=P)
    epool = ctx.enter_context(tc.tile_pool(name="epool", bufs=52))
    small = ctx.enter_context(tc.tile_pool(name="small", bufs=8))
    outp = ctx.enter_context(tc.tile_pool(name="outp", bufs=4))
    usb_pool = ctx.enter_context(tc.tile_pool(name="usb", bufs=1))

    # scores PSUM pool first so it does NOT share banks with (and thus wait
    # on) the projection pool: 4 + 3 banks coexist, u/sum pools reuse the
    # proj banks after release (they wait for the V gather anyway).
    pp_s = ctx.enter_context(tc.tile_pool(name="pp_s", space="PSUM", bufs=2))

    # interleaved (g0, ch)(g1, ch) score+exp pair emission, fed just-in-time
    # as kT blocks complete so PE FIFO order never head-of-line blocks ACT
    es = {g: [] for g in range(NGROUPS)}
    # group 0 leads group 1 by LEAD chunks so the very first exps never
    # head-of-line block on q1's projection (which follows the first kacc)
    LEAD = 1
    SCHED = [(0, c) for c in range(LEAD)]
    for c in range(NKCH):
        if c + LEAD < NKCH:
            SCHED.append((0, c + LEAD))
        SCHED.append((1, c))
    _cursor = [0]  # position in SCHED (2 entries per chunk overall)

    def emit_one(g, ch):
        q0 = g * GW
        s_ps = pp_s.tile([P, 2 * GW], F32, tag="s", name="s_ps")
        nc.tensor.matmul(
            s_ps[:, 0:GW],
            kT_sb[0:H2, ch * P : (ch + 1) * P],
            qT_sb[0:H2, q0 : q0 + GW],
        )
        nc.tensor.matmul(
            s_ps[:, GW : 2 * GW],
            kT_sb[H2:H, ch * P : (ch + 1) * P],
            qT_sb[H2:H, q0 : q0 + GW],
        )
        e_sb = epool.tile([P, 2 * GW], BF16, tag="e", name="e_sb")
        nc.scalar.activation(e_sb, s_ps, AF.Exp, scale=SCALE)
        es[g].append(e_sb)

    def emit_pairs(limit_ch, maxn):
        # maxn counts chunk-equivalents (2 SCHED entries)
        n = 2 * maxn
        while _cursor[0] < len(SCHED) and n > 0:
            g, ch = SCHED[_cursor[0]]
            if ch >= limit_ch:
                break
            emit_one(g, ch)
            _cursor[0] += 1
            n -= 1

    # ---- projections (x streamed as half-block tiles for tight deps) ----
    # DMA order: x0h0, wv, x0h1, wk, wq, x1h0, x1h1, x2... so the V->collective
    # gate (x0+wv) and the first-kT gate (wk+x1h0) resolve as early as possible.
    with tc.tile_pool(name="xstream", bufs=3) as xpool, \
         tc.tile_pool(name="pp_proj", space="PSUM", bufs=1) as pp_proj:

        def load_xh(pos, sl, quarters=False):
            xh = xpool.tile([P, DCH, 512], BF16, tag="x", name=f"x{pos}_{sl}")
            if quarters:
                for qq in range(4):
                    nc.sync.dma_start(
                        out=xh[:, qq * 4 : (qq + 1) * 4, :],
                        in_=xr[pos, :, qq * 4 : (qq + 1) * 4,
                               sl * 512 : (sl + 1) * 512],
                    )
            else:
                nc.sync.dma_start(
                    out=xh, in_=xr[pos, :, :, sl * 512 : (sl + 1) * 512]
                )
            return xh

        nc.sync.dma_start(out=wv_sb, in_=wvT)
        x0h = [load_xh(0, 0, quarters=True)]
        nc.sync.dma_start(out=wq_sb, in_=wqT)
        x0h.append(load_xh(0, 1, quarters=True))
        nc.sync.dma_start(out=wk_sb, in_=wkT)

        # PE p-state warm-up: the tensor engine only reaches 2.4GHz after
        # 3us of CONTINUOUS execution, and any idle gap resets the ramp.
        # Run discarded matmuls from t~0 so the V projection (which gates
        # the collective) starts at full clock with the ramp already paid.
        dummy_sb = consts.tile([P, 512], BF16)
        nc.vector.memset(dummy_sb, 0.0)

        def warm(n):
            s_dummy = pp_s.tile([P, 2 * GW], F32, tag="s", name="warm")
            for _ in range(n):
                nc.tensor.matmul(s_dummy[0:1, 0:512], ones_bf, dummy_sb)


        # V (own block): stationary x chunk [d,128s], moving wv [d,128h].
        # Both V halves back-to-back: the AllGather dispatch is the kernel's
        # longest gate (collective ~41us + B-phase ~28us behind it).
        def proj_q(hf2):
            qacc = pp_proj.tile([P, 512], F32, tag="kacc", bufs=2)
            for c in range(DCH):
                nc.tensor.matmul(
                    qacc, wq_sb[:, c, :], x0h[hf2][:, c, :],
                    start=(c == 0), stop=(c == DCH - 1),
                )
            nc.vector.tensor_copy(qT_sb[:, hf2 * 512 : (hf2 + 1) * 512], qacc)

        for hf in range(2):
            vacc = pp_proj.tile([P, 4, P], F32, tag="vacc", bufs=1)
            for j4 in range(4):
                for c in range(DCH):
                    nc.tensor.matmul(
                        vacc[:, j4, :],
                        x0h[hf][:, c, j4 * P : (j4 + 1) * P],
                        wv_sb[:, c, :],
                        start=(c == 0),
                        stop=(c == DCH - 1),
                    )
            nc.vector.tensor_copy(vpart_sb[:, hf * 4 : (hf + 1) * 4, :], vacc)
            if hf == 0:
                proj_q(0)  # fills the PE gap while x0h1 streams in
        nc.sync.dma_start(out=part_d, in_=vpart_sb.rearrange("p j h -> p (j h)"))
        nc.gpsimd.collective_compute(
            "AllGather",
            OP.bypass,
            replica_groups=[[0, 1, 2, 3], [4, 5, 6, 7]],
            ins=[part_d.opt()],
            outs=[full_d.opt()],
        )


        # kT for batch blocks 0..3 (stream positions 1..4) with score/exp
        # pairs interleaved as coverage grows
        for blk in range(NBLK):
            for sl in range(2):
                xh = load_xh(blk + 1, sl, quarters=(blk == 0))
                kacc = pp_proj.tile([P, 512], F32, tag="kacc", bufs=2)
                for c in range(DCH):
                    nc.tensor.matmul(
                        kacc, wk_sb[:, c, :], xh[:, c, :],
                        start=(c == 0), stop=(c == DCH - 1),
                    )
                nc.vector.tensor_copy(
                    kT_sb[:, blk * BLKW + sl * 512 : blk * BLKW + (sl + 1) * 512],
                    kacc,
                )
                if blk == 0 and sl == 0:
                    proj_q(1)
                emit_pairs(blk * 8 + (sl + 1) * 4, 5)

    # gathered V -> SBUF (batch order: host maps rank r -> block order)
    for r in range(NBLK):
        nc.sync.dma_start(
            out=v_sb[:, r * 8 : (r + 1) * 8, :],
            in_=full_d[r].rearrange("p (j h) -> p j h", j=8),
        )

    pp_u = ctx.enter_context(tc.tile_pool(name="pp_u", space="PSUM", bufs=1))
    pp_sum = ctx.enter_context(tc.tile_pool(name="pp_sum", space="PSUM", bufs=1))
    pp_u1h = ctx.enter_context(tc.tile_pool(name="pp_u1h", space="PSUM", bufs=1))

    # ---- attention ----
    # Phase A (both groups back-to-back so ACT never starves): scores + exp
    # for all 64 chunk-visits; e is buffered in SBUF.
    # Phase B (per group, after the V gather lands): PV + sums bursts via
    # e-stationary matmuls (sums outputs have free-size 1 => ~free).
    c_ = 1.0 - LAMBDA_INIT
    a_ = 1.0 / (H * c_ * c_)
    b_ = RMS_EPS / (c_ * c_)
    bias_b = consts.tile([P, 1], F32)
    nc.vector.memset(bias_b, b_)

    def bcast_h(t, j0):
        """[P,1] columns j0..j0+NJ of t broadcast along h: [P, NJ, P]"""
        base = t.opt()
        return bass.AP(
            tensor=t.tensor,
            offset=base.offset + j0,
            ap=[list(base.ap[0]), [1, NJ], [0, P]],
        )

    def u_of(g, u_ap, hh, j):
        if u_ap is not None:
            return u_ap[:, hh * NJ + j, :]
        return get_u(1, hh)[:, j, :]

    def post_group(g, u_ap, sums_ps):
        ve = nc.vector
        """normalize + combine + RMS + store for one group (u_ap may be
        PSUM or SBUF). Batched across the NJ q-subtiles."""
        r_sb = small.tile([P, 2 * NJ], F32, tag="r", bufs=2)
        nc.vector.reciprocal(r_sb, sums_ps)
        nc.vector.tensor_scalar_mul(r_sb[:, NJ:], r_sb[:, NJ:], lam)
        # attn[j] = u1[j]*r1[j] - u2[j]*r2[j]: per-j fused stt chains
        # (one tensor_scalar_mul + one scalar_tensor_tensor per subtile --
        # shorter serial chain than batched wide ops)
        t2 = small.tile([P, NJ, P], F32, tag="t2", bufs=2)
        attn_sb = outp.tile([P, NJ, P], F32, tag="attn", bufs=2)
        for j in range(NJ):
            ve.tensor_scalar_mul(
                t2[:, j, :], u_of(g, u_ap, 1, j), r_sb[:, NJ + j : NJ + j + 1]
            )
            nc.vector.scalar_tensor_tensor(
                attn_sb[:, j, :], u_of(g, u_ap, 0, j), r_sb[:, j : j + 1],
                t2[:, j, :], op0=OP.mult, op1=OP.subtract,
            )
        # RMS stats: per-j accumulated square sums into one [P, NJ] tile
        ssq = small.tile([P, NJ], F32, tag="ssq", bufs=2)
        sq_scr = small.tile([P, NJ, P], F32, tag="sqscr", bufs=2)
        for j in range(NJ):
            nc.scalar.activation(
                sq_scr[:, j, :], attn_sb[:, j, :], AF.Square,
                accum_out=ssq[:, j : j + 1],
            )
        root = small.tile([P, NJ], F32, tag="root", bufs=2)
        nc.scalar.activation(root, ssq, AF.Sqrt, scale=a_, bias=bias_b)
        rrms = small.tile([P, NJ], F32, tag="rrms", bufs=2)
        nc.vector.reciprocal(rrms, root)
        # batched store: o[p, j, h] = attn * rrms (bcast h) * rmsw (bcast j),
        # one wide DVE chain and ONE output DMA per group
        o_sb = outp.tile([P, NJ, H], F32, tag="o", bufs=1)
        for j in range(NJ):
            nc.vector.scalar_tensor_tensor(
                o_sb[:, j, :], attn_sb[:, j, :], rrms[:, j : j + 1], rmsw_bc,
                op0=OP.mult, op1=OP.mult,
            )
        nc.sync.dma_start(
            out=bass.AP(
                tensor=out_d.tensor,
                offset=g * GW * H,
                ap=[[H, P], [P * H, NJ], [1, H]],
            ),
            in_=o_sb,
        )

    # Phase B. ch-outer / hj-inner with ONE accumulation group per 2KB psum
    # zero region: u bank0 = cells hj 0..3, u bank1 = hj 4..7, sums = all 8
    # cells in one bank. start fires on the region's first write, stop on
    # its last. B(g0) chunks are interleaved into the late score pairs (see
    # emit loop below) so they don't serialize behind the score tail.
    # PSUM bank plan after proj releases (4 free banks): u0 (2 banks) +
    # shared sums (1 bank, all 16 cells, ONE accumulation group spanning the
    # whole B phase) + u1-half1 (1 bank) accumulate CONCURRENTLY; u1-half2
    # reuses u0's slot after the copy-out. This halves the serialized B1
    # tail that used to run entirely after B0 closed.
    b_tiles = {}
    scount = [0]
    SUMS_TOTAL = 2 * NKCH * 2 * NJ

    def get_sums():
        if "s" not in b_tiles:
            b_tiles["s"] = pp_sum.tile([P, 4 * NJ], F32, tag="sum",
                                       name="sums", bufs=1)
        return b_tiles["s"]

    def get_u(g, hh):
        # u0: one [P, 8, P] tile (2 banks, both halves). u1h1: its own
        # 1-bank tile. u1h2: a fresh tag-u tile (waits for u0's release).
        key = (g, hh)
        if key not in b_tiles:
            if g == 0:
                t = b_tiles.get((0, 0))
                if t is None:
                    t = pp_u.tile([P, 2 * NJ, P], F32, tag="u", name="u0")
                b_tiles[(0, 0)] = t
                b_tiles[(0, 1)] = t
            elif hh == 0:
                b_tiles[key] = pp_u1h.tile([P, NJ, P], F32, tag="u1h",
                                           name="u1h1", bufs=1)
            else:
                b_tiles[key] = pp_u.tile([P, 2 * NJ, P], F32, tag="u",
                                         name="u1h2")
        return b_tiles[(g, hh)]

    def emit_B(g, hh, ch):
        u_ps = get_u(g, hh)
        sums_ps = get_sums()
        for j in range(NJ):
            hj = hh * NJ + j
            e_sub = es[g][ch][:, hj * P : (hj + 1) * P]
            if g == 0:
                u_slice = u_ps[:, hj, :]
            else:
                u_slice = u_ps[:, j, :]
            nc.tensor.matmul(
                u_slice, e_sub, v_sb[:, ch, :],
                start=(ch == 0 and j == 0),
                stop=(ch == NKCH - 1 and j == NJ - 1),
                skip_group_check=True,
            )
            nc.tensor.matmul(
                sums_ps[:, g * 2 * NJ + hj : g * 2 * NJ + hj + 1],
                e_sub, ones_bf,
                start=(scount[0] == 0),
                stop=(scount[0] == SUMS_TOTAL - 1),
                skip_group_check=True,
            )
            scount[0] += 1

    # three B work queues: q0 = B0 chunks (both halves, 2 banks), q1 = B1
    # half1 (own bank, CONCURRENT with B0), q2 = B1 half2 (after u0 frees).
    bcur = [0, 0, 0]
    g0_closed = [False]

    def close_g0():
        u_sb = usb_pool.tile([P, 2 * NJ, P], F32, tag="usb")
        nc.vector.tensor_copy(u_sb, get_u(0, 0))
        post_group(0, u_sb, get_sums()[:, 0 : 2 * NJ])
        g0_closed[0] = True

    def emit_B0(ch):
        emit_B(0, 0, ch)
        emit_B(0, 1, ch)

    def maybe_B(n):
        while n > 0 and _cursor[0] >= 38:
            did = False
            if bcur[0] < NKCH and bcur[0] < len(es[0]):
                emit_B0(bcur[0])
                bcur[0] += 1
                n -= 1
                did = True
            if n > 0 and bcur[1] < NKCH and bcur[1] < len(es[1]):
                emit_B(1, 0, bcur[1])
                bcur[1] += 1
                n -= 1
                did = True
            if (n > 0 and bcur[0] >= NKCH and bcur[2] < NKCH
                    and bcur[2] < len(es[1])):
                if not g0_closed[0]:
                    close_g0()
                emit_B(1, 1, bcur[2])
                bcur[2] += 1
                n -= 1
                did = True
            if not did:
                break

    while _cursor[0] < len(SCHED):
        emit_pairs(NKCH, 1)
        maybe_B(4)
    while bcur[0] < NKCH:
        emit_B0(bcur[0])
        bcur[0] += 1
    while bcur[1] < NKCH:
        emit_B(1, 0, bcur[1])
        bcur[1] += 1
    if not g0_closed[0]:
        close_g0()
    while bcur[2] < NKCH:
        emit_B(1, 1, bcur[2])
        bcur[2] += 1
    post_group(1, None, get_sums()[:, 2 * NJ : 4 * NJ])


def build(lam: float):
    from concourse._compat import axon_active

    nc = bacc.Bacc(
        "TRN2",
        target_bir_lowering=False,
        debug=not axon_active(),
        num_devices=NCORES,
    )
    with tile.TileContext(nc) as tc:
        with ExitStack() as ctx:
            _emit(ctx, tc, lam)
    nc.compile()
    return nc


def make_in_maps(x, Wq, Wk, Wv, rms_weight):
    bf = ml_dtypes.bfloat16
    x = np.asarray(x, dtype=np.float32)
    xT = np.ascontiguousarray(x.transpose(0, 2, 1)).astype(bf)  # [B, D, S]
    def warr(W):
        # [P, DCH, H]: warr[p, c, h] = W.T[c*128 + p, h]
        wT = np.asarray(W, np.float32).T.reshape(DCH, P, H)
        return np.ascontiguousarray(wT.transpose(1, 0, 2)).astype(bf)

    wqT, wkT, wvT = warr(Wq), warr(Wk), warr(Wv)
    rw = np.ascontiguousarray(np.asarray(rms_weight, np.float32))
    in_maps = []
    for core in range(NCORES):
        b, qb = divmod(core, NCORES // B)
        # 5 xT blocks: position 0 = own block (feeds V/q projections),
        # positions 1..4 = batch blocks 0..3 (feed kT in batch order, which
        # matches the V AllGather rank order). The duplicate own block keeps
        # the SPMD program free of per-core block-index knowledge.
        blocks = [xT[b][:, qb * BLKW : (qb + 1) * BLKW]]
        for r in range(NBLK):
            blocks.append(xT[b][:, r * BLKW : (r + 1) * BLKW])
        in_maps.append(
            {
                "xT": np.ascontiguousarray(np.stack(blocks)),
                "wqT": wqT,
                "wkT": wkT,
                "wvT": wvT,
                "rmsw": rw,
            }
        )
    return in_maps


def kernel(x, Wq, Wk, Wv, lambda_q1, lambda_q2, lambda_k1, lambda_k2, rms_weight):
    lq1 = np.asarray(lambda_q1, np.float32)
    lq2 = np.asarray(lambda_q2, np.float32)
    lk1 = np.asarray(lambda_k1, np.float32)
    lk2 = np.asarray(lambda_k2, np.float32)
    lam = float(np.exp(np.dot(lq1, lk1)) - np.exp(np.dot(lq2, lk2)) + LAMBDA_INIT)
    nc = build(lam)
    in_maps = make_in_maps(x, Wq, Wk, Wv, rms_weight)
    res = bass_utils.run_bass_kernel_spmd(nc, in_maps, core_ids=list(range(NCORES)))
    out = np.empty((B, S, H), np.float32)
    for core in range(NCORES):
        b, qb = divmod(core, NCORES // B)
        out[b, qb * QSHARD : (qb + 1) * QSHARD] = res.results[core]["out"]
    return out
